# revision 8
# baseline (speedup 1.0000x reference)
"""MoE (top-2 of 8 experts) Trainium2 kernel, expert-parallel across 8 cores.

Strategy (per core e):
  - replicate x, gate weights; core e holds expert e's W1/b1/W2/b2 (bf16 FFN
    weights, fp32 gate).
  - gate computed on-device in fp32 (PE transpose of x tiles + PE matmul,
    softmax on ACT/DVE). top-2 selection via DVE sort-8 (vector.max).
  - stream compaction of selected tokens per 1024-token chunk with a fixed
    capacity of CAP=384: prefix-sums via triangular matmuls, compacted
    (token_idx, coef, occupancy) via a one-hot permutation matmul.
  - indirect-DMA gather of selected x rows, bf16 FFN (relu MLP), scale by
    gate coef, indirect-DMA scatter into a zeroed partial buffer.
  - per-chunk ReduceScatter(add) over all 8 cores combines the two expert
    contributions per token; host just concatenates the shards.
"""

import numpy as np
import ml_dtypes

B, L, D, DFF, E = 2, 2048, 1024, 4096, 8
N = B * L                # 4096 tokens
P = 128
KD = D // P              # 8   contraction chunks over D
NDJ = DFF // P           # 32  DFF tiles
NCHUNK = 4
CHUNK = N // NCHUNK      # 1024 tokens per chunk
TPC = CHUNK // P         # 8   token tiles per chunk
CAP = 384                # per-expert capacity per chunk
SG = CAP // P            # 3   slot groups
N_CORES = 8
HALF = D // 2            # 512

_cache = {}


def _build():
    import concourse.bass as bass
    import concourse.mybir as mybir
    import concourse.tile as tile
    from concourse import bacc
    from concourse.masks import make_identity

    dt = mybir.dt
    AF = mybir.ActivationFunctionType
    OP = mybir.AluOpType

    nc = bacc.Bacc("TRN2", target_bir_lowering=False, debug=False,
                   num_devices=N_CORES)

    # ---- kernel I/O ----
    x_d = nc.dram_tensor("x", [N, D], dt.float32, kind="ExternalInput")
    w1_d = nc.dram_tensor("w1", [P, KD, DFF], dt.bfloat16, kind="ExternalInput")
    w2_d = nc.dram_tensor("w2", [P, NDJ, D], dt.bfloat16, kind="ExternalInput")
    b1_d = nc.dram_tensor("b1", [P, NDJ], dt.float32, kind="ExternalInput")
    b2_d = nc.dram_tensor("b2", [1, D], dt.float32, kind="ExternalInput")
    wg_d = nc.dram_tensor("wg", [P, KD, E], dt.float32, kind="ExternalInput")
    bg_d = nc.dram_tensor("bg", [P, E], dt.float32, kind="ExternalInput")
    sel_d = nc.dram_tensor("sel", [P, E], dt.float32, kind="ExternalInput")
    lst_d = nc.dram_tensor("lst", [P, P], dt.float32, kind="ExternalInput")
    ust_d = nc.dram_tensor("ust", [E, E], dt.float32, kind="ExternalInput")
    slot_d = nc.dram_tensor("slot", [P, CAP], dt.float32, kind="ExternalInput")
    iota_d = nc.dram_tensor("iota", [P, 1], dt.float32, kind="ExternalInput")
    ones1_d = nc.dram_tensor("ones1", [1, P], dt.float32, kind="ExternalInput")

    out_d = nc.dram_tensor("out_shard", [NCHUNK, P, D], dt.float32,
                           kind="ExternalOutput")

    rg = [list(range(N_CORES))]

    with tile.TileContext(nc) as tc:
        with (
            tc.tile_pool(name="const", bufs=1) as const,
            tc.tile_pool(name="xpool", bufs=3) as xpool,
            tc.tile_pool(name="xtpool", bufs=2) as xtpool,
            tc.tile_pool(name="xgpool", bufs=4) as xgpool,
            tc.tile_pool(name="hpool", bufs=1) as hpool,
            tc.tile_pool(name="w2pool", bufs=4) as w2pool,
            tc.tile_pool(name="ypool", bufs=4) as ypool,
            tc.tile_pool(name="ppool", bufs=2) as ppool,
            tc.tile_pool(name="spool", bufs=3) as spool,
            tc.tile_pool(name="chpool", bufs=4) as chpool,
            tc.tile_pool(name="psum", bufs=1, space="PSUM") as psum,
            tc.tile_pool(name="dram", bufs=1, space="DRAM") as dram,
        ):
            # ---------- constants ----------
            ident = const.tile([P, P], dt.float32, tag="ident")
            make_identity(nc, ident[:])
            w1sb = const.tile([P, KD, DFF], dt.bfloat16, tag="w1sb")
            nc.sync.dma_start(w1sb[:], w1_d[:])
            b1sb = const.tile([P, NDJ], dt.float32, tag="b1sb")
            nc.sync.dma_start(b1sb[:], b1_d[:])
            wgsb = const.tile([P, KD, E], dt.float32, tag="wgsb")
            nc.sync.dma_start(wgsb[:], wg_d[:])
            bgsb = const.tile([P, E], dt.float32, tag="bgsb")
            nc.sync.dma_start(bgsb[:], bg_d[:])
            selsb = const.tile([P, E], dt.float32, tag="selsb")
            nc.sync.dma_start(selsb[:], sel_d[:])
            lst = const.tile([P, P], dt.float32, tag="lst")
            nc.sync.dma_start(lst[:], lst_d[:])
            ust = const.tile([E, E], dt.float32, tag="ust")
            nc.sync.dma_start(ust[:], ust_d[:])
            slotsb = const.tile([P, CAP], dt.float32, tag="slotsb")
            nc.sync.dma_start(slotsb[:], slot_d[:])
            iotasb = const.tile([P, 1], dt.float32, tag="iotasb")
            nc.sync.dma_start(iotasb[:], iota_d[:])
            ones1sb = const.tile([1, P], dt.float32, tag="ones1sb")
            nc.sync.dma_start(ones1sb[:], ones1_d[:])
            b2row = const.tile([1, D], dt.float32, tag="b2row")
            nc.sync.dma_start(b2row[:], b2_d[:])

            # broadcast b2 across partitions via K=1 matmul
            b2b = const.tile([P, D], dt.float32, tag="b2b")
            for h in range(2):
                pb = psum.tile([P, HALF], dt.float32, tag="pmlp2", bufs=3)
                nc.tensor.matmul(pb[:], lhsT=ones1sb[:, :],
                                 rhs=b2row[:, h * HALF:(h + 1) * HALF],
                                 start=True, stop=True)
                nc.vector.tensor_copy(b2b[:, h * HALF:(h + 1) * HALF], pb[:])

            # zero tile for clearing partial buffers
            zt = const.tile([P, D], dt.float32, tag="zt")
            nc.vector.memset(zt[:], 0.0)

            # internal DRAM: per-chunk partial + RS output
            partials = []
            rs_outs = []
            for c in range(NCHUNK):
                pc = dram.tile([CHUNK + 8, D], dt.float32, tag=f"partial{c}")
                partials.append(pc)
                ro = dram.tile([P, D], dt.float32, tag=f"rsout{c}")
                rs_outs.append(ro)
                for i in range(TPC):
                    nc.scalar.dma_start(pc[i * P:(i + 1) * P, :], zt[:])

            # ---------- phase 1: gate + compaction + gather (all chunks) ----
            idx_g_all = []   # per chunk: [P, SG] int32 gather indices (global)
            idx_s_all = []   # per chunk: [P, SG] int32 scatter indices (local)
            coef_all = []    # per chunk: [P, SG] f32 gate coefficients
            xgT_all = []     # per chunk: [P, KD, CAP] bf16 gathered tokens^T

            for c in range(NCHUNK):
                mask_ch = chpool.tile([P, TPC], dt.float32, tag="mask")
                coef_ch = chpool.tile([P, TPC], dt.float32, tag="coef")

                for f in range(TPC):
                    r0 = (c * TPC + f) * P
                    xa = xpool.tile([P, D], dt.float32, tag="xa", bufs=2)
                    nc.sync.dma_start(xa[:], x_d[r0:r0 + P, :])
                    xT = xtpool.tile([P, KD, P], dt.float32, tag="xT")
                    for kc in range(KD):
                        pt = psum.tile([P, P], dt.float32, tag="ptrans", bufs=2)
                        nc.tensor.transpose(pt[:], xa[:, kc * P:(kc + 1) * P],
                                            ident[:])
                        nc.vector.tensor_copy(xT[:, kc, :], pt[:])
                    pg = psum.tile([P, E], dt.float32, tag="pgate", bufs=1)
                    for kc in range(KD):
                        nc.tensor.matmul(pg[:], lhsT=xT[:, kc, :],
                                         rhs=wgsb[:, kc, :],
                                         start=(kc == 0), stop=(kc == KD - 1))
                    logits = spool.tile([P, E], dt.float32, tag="logits")
                    nc.vector.tensor_add(logits[:], pg[:], bgsb[:])
                    srt = spool.tile([P, E], dt.float32, tag="srt")
                    nc.vector.max(srt[:], logits[:])
                    tmp8 = spool.tile([P, E], dt.float32, tag="tmp8")
                    nc.vector.tensor_mul(tmp8[:], logits[:], selsb[:])
                    lour = spool.tile([P, 1], dt.float32, tag="lour")
                    nc.vector.reduce_sum(lour[:], tmp8[:],
                                         axis=mybir.AxisListType.X)
                    m1 = spool.tile([P, 1], dt.float32, tag="m1")
                    nc.vector.reduce_max(m1[:], logits[:],
                                         axis=mybir.AxisListType.X)
                    negm = spool.tile([P, 1], dt.float32, tag="negm")
                    nc.vector.tensor_scalar_mul(negm[:], m1[:], -1.0)
                    exps = spool.tile([P, E], dt.float32, tag="exps")
                    nc.scalar.activation(exps[:], logits[:], AF.Exp,
                                         bias=negm[:, 0:1])
                    ssum = spool.tile([P, 1], dt.float32, tag="ssum")
                    nc.vector.reduce_sum(ssum[:], exps[:],
                                         axis=mybir.AxisListType.X)
                    rinv = spool.tile([P, 1], dt.float32, tag="rinv")
                    nc.vector.reciprocal(rinv[:], ssum[:])
                    tmp8b = spool.tile([P, E], dt.float32, tag="tmp8b")
                    nc.vector.tensor_mul(tmp8b[:], exps[:], selsb[:])
                    eour = spool.tile([P, 1], dt.float32, tag="eour")
                    nc.vector.reduce_sum(eour[:], tmp8b[:],
                                         axis=mybir.AxisListType.X)
                    # selected iff our logit >= 2nd-largest logit
                    nc.vector.tensor_tensor(mask_ch[:, f:f + 1], lour[:],
                                            srt[:, 1:2], op=OP.is_ge)
                    cv = spool.tile([P, 1], dt.float32, tag="cv")
                    nc.vector.tensor_mul(cv[:], eour[:], rinv[:])
                    nc.vector.tensor_mul(coef_ch[:, f:f + 1], cv[:],
                                         mask_ch[:, f:f + 1])

                # ----- compaction -----
                # column (=tile) totals: transpose mask -> [TPC, P], row-sum
                mt_ps = psum.tile([P, P], dt.float32, tag="pgate", bufs=1)
                nc.tensor.transpose(mt_ps[:TPC, :], mask_ch[:], ident[:])
                mts = spool.tile([TPC, P], dt.float32, tag="mts")
                nc.vector.tensor_copy(mts[:], mt_ps[:TPC, :])
                cs = spool.tile([TPC, 1], dt.float32, tag="cs")
                nc.vector.reduce_sum(cs[:], mts[:], axis=mybir.AxisListType.X)
                cs_b = spool.tile([TPC, P], dt.float32, tag="cs_b")
                nc.vector.tensor_copy(cs_b[:], cs[:].to_broadcast([TPC, P]))
                # pos[p,f] = (# selected with q<p in tile f) + (# selected in
                # tiles g<f)  — two accumulated matmuls
                ppos = psum.tile([P, E], dt.float32, tag="pgate", bufs=1)
                nc.tensor.matmul(ppos[:, :TPC], lhsT=lst[:], rhs=mask_ch[:],
                                 start=True, stop=False)
                nc.tensor.matmul(ppos[:, :TPC], lhsT=cs_b[:], rhs=ust[:],
                                 start=False, stop=True)
                # pos_eff = mask ? pos : CAP
                t1 = spool.tile([P, TPC], dt.float32, tag="t1")
                nc.vector.tensor_scalar_add(t1[:], ppos[:, :TPC], -float(CAP))
                t2 = spool.tile([P, TPC], dt.float32, tag="t2")
                nc.vector.tensor_mul(t2[:], t1[:], mask_ch[:])
                pos_eff = chpool.tile([P, TPC], dt.float32, tag="pos_eff")
                nc.vector.tensor_scalar_add(pos_eff[:], t2[:], float(CAP))

                # permutation matmuls -> compact [idx, coef, occ]
                pcmp = psum.tile([P, 3 * SG], dt.float32, tag="pacc", bufs=2)
                for f in range(TPC):
                    perm = ppool.tile([P, CAP], dt.float32, tag="perm")
                    nc.vector.tensor_tensor(
                        perm[:], pos_eff[:, f:f + 1].to_broadcast([P, CAP]),
                        slotsb[:], op=OP.is_equal)
                    rhs3 = spool.tile([P, 3], dt.float32, tag="rhs3")
                    nc.vector.tensor_scalar_add(rhs3[:, 0:1], iotasb[:],
                                                float(f * P))
                    nc.vector.tensor_copy(rhs3[:, 1:2], coef_ch[:, f:f + 1])
                    nc.vector.memset(rhs3[:, 2:3], 1.0)
                    for sg in range(SG):
                        # one zero-region: only the very first matmul starts
                        # the accumulation group; pending-zero covers the
                        # other slot-group slices of the same PSUM bank.
                        nc.tensor.matmul(pcmp[:, 3 * sg:3 * sg + 3],
                                         lhsT=perm[:, sg * P:(sg + 1) * P],
                                         rhs=rhs3[:],
                                         start=(f == 0 and sg == 0),
                                         stop=(f == TPC - 1
                                               and sg == SG - 1))

                idx_g_i = chpool.tile([P, SG], dt.int32, tag="idx_g")
                idx_s_i = chpool.tile([P, SG], dt.int32, tag="idx_s")
                coef_sg = chpool.tile([P, SG], dt.float32, tag="coef_sg")
                for sg in range(SG):
                    cmp = spool.tile([P, 3], dt.float32, tag="cmp")
                    nc.vector.tensor_copy(cmp[:], pcmp[:, 3 * sg:3 * sg + 3])
                    nc.vector.tensor_copy(coef_sg[:, sg:sg + 1], cmp[:, 1:2])
                    gidx = spool.tile([P, 1], dt.float32, tag="gidx")
                    nc.vector.tensor_scalar_add(gidx[:], cmp[:, 0:1],
                                                float(c * CHUNK))
                    nc.vector.tensor_copy(idx_g_i[:, sg:sg + 1], gidx[:])
                    # scatter idx: local idx, empty slots -> trash row CHUNK
                    iv = spool.tile([P, 1], dt.float32, tag="iv")
                    nc.vector.tensor_scalar(iv[:], cmp[:, 2:3], -float(CHUNK),
                                            float(CHUNK), op0=OP.mult,
                                            op1=OP.add)
                    sidx = spool.tile([P, 1], dt.float32, tag="sidx")
                    nc.vector.tensor_add(sidx[:], cmp[:, 0:1], iv[:])
                    nc.vector.tensor_copy(idx_s_i[:, sg:sg + 1], sidx[:])

                # ----- gather + transpose (fp32 -> bf16) -----
                xgT = xgpool.tile([P, KD, CAP], dt.bfloat16, tag="xgT")
                for sg in range(SG):
                    xg = xpool.tile([P, D], dt.float32, tag="xg")
                    nc.gpsimd.indirect_dma_start(
                        out=xg[:], out_offset=None, in_=x_d[:, :],
                        in_offset=bass.IndirectOffsetOnAxis(
                            ap=idx_g_i[:, sg:sg + 1], axis=0))
                    for kc in range(KD):
                        pt = psum.tile([P, P], dt.float32, tag="ptrans",
                                       bufs=2)
                        nc.tensor.transpose(pt[:], xg[:, kc * P:(kc + 1) * P],
                                            ident[:])
                        nc.vector.tensor_copy(
                            xgT[:, kc, sg * P:(sg + 1) * P], pt[:])

                idx_g_all.append(idx_g_i)
                idx_s_all.append(idx_s_i)
                coef_all.append(coef_sg)
                xgT_all.append(xgT)

            # ---------- phase 2: FFN + scatter + reduce-scatter ----------
            for c in range(NCHUNK):
                xgT = xgT_all[c]
                idx_s_i = idx_s_all[c]
                coef_sg = coef_all[c]

                hT = hpool.tile([P, NDJ, CAP], dt.bfloat16, tag="hT")
                for dj in range(NDJ):
                    ph = psum.tile([P, CAP], dt.float32, tag="pacc", bufs=2)
                    for kc in range(KD):
                        nc.tensor.matmul(
                            ph[:], lhsT=w1sb[:, kc, dj * P:(dj + 1) * P],
                            rhs=xgT[:, kc, :],
                            start=(kc == 0), stop=(kc == KD - 1))
                    nc.scalar.activation(hT[:, dj, :], ph[:], AF.Relu,
                                         bias=b1sb[:, dj:dj + 1])

                youts = [ypool.tile([P, D], dt.float32, tag="yout",
                                     name=f"yout{c}_{i}") for i in range(SG)]
                for h in range(2):
                    pys = [psum.tile([P, HALF], dt.float32, tag="pmlp2",
                                     bufs=3, name=f"py{c}_{h}_{i}")
                           for i in range(SG)]
                    for dj in range(NDJ):
                        w2t = w2pool.tile([P, HALF], dt.bfloat16, tag="w2t")
                        nc.sync.dma_start(
                            w2t[:], w2_d[:, dj, h * HALF:(h + 1) * HALF])
                        for sg in range(SG):
                            nc.tensor.matmul(
                                pys[sg][:],
                                lhsT=hT[:, dj, sg * P:(sg + 1) * P],
                                rhs=w2t[:],
                                start=(dj == 0), stop=(dj == NDJ - 1))
                    for sg in range(SG):
                        hs = slice(h * HALF, (h + 1) * HALF)
                        nc.vector.tensor_add(youts[sg][:, hs], pys[sg][:],
                                             b2b[:, hs])
                        nc.vector.tensor_scalar_mul(youts[sg][:, hs],
                                                    youts[sg][:, hs],
                                                    coef_sg[:, sg:sg + 1])
                for sg in range(SG):
                    nc.gpsimd.indirect_dma_start(
                        out=partials[c][:, :],
                        out_offset=bass.IndirectOffsetOnAxis(
                            ap=idx_s_i[:, sg:sg + 1], axis=0),
                        in_=youts[sg][:], in_offset=None)

                nc.gpsimd.collective_compute(
                    "ReduceScatter", mybir.AluOpType.add, replica_groups=rg,
                    ins=[partials[c][0:CHUNK, :].opt()],
                    outs=[rs_outs[c][:, :].opt()])
                nc.sync.dma_start(out_d[c, :, :], rs_outs[c][:, :])

    nc.compile()
    return nc


def _host_inputs(x, W1, b1, W2, b2, Wg, bg):
    bf16 = ml_dtypes.bfloat16
    f32 = np.float32
    x2 = np.ascontiguousarray(x.reshape(N, D), dtype=f32)
    lst = np.triu(np.ones((P, P), f32), k=1)       # lst[q, m] = 1 if q < m
    ust = np.triu(np.ones((E, E), f32), k=1)       # ust[g, f] = 1 if g < f
    slot = np.tile(np.arange(CAP, dtype=f32), (P, 1))
    iota = np.arange(P, dtype=f32).reshape(P, 1)
    ones1 = np.ones((1, P), f32)
    in_maps = []
    for e in range(N_CORES):
        sel = np.zeros((E,), f32)
        sel[e] = 1.0
        in_maps.append({
            "x": x2,
            "w1": np.ascontiguousarray(
                W1[e].reshape(KD, P, DFF).transpose(1, 0, 2)).astype(bf16),
            "w2": np.ascontiguousarray(
                W2[e].reshape(NDJ, P, D).transpose(1, 0, 2)).astype(bf16),
            "b1": np.ascontiguousarray(
                b1[e].reshape(NDJ, P).T).astype(f32),
            "b2": b2[e].reshape(1, D).astype(f32),
            "wg": np.ascontiguousarray(
                Wg.reshape(KD, P, E).transpose(1, 0, 2)).astype(f32),
            "bg": np.tile(bg.astype(f32), (P, 1)),
            "sel": np.tile(sel, (P, 1)),
            "lst": lst, "ust": ust, "slot": slot, "iota": iota,
            "ones1": ones1,
        })
    return in_maps


def _assemble(results):
    out = np.empty((N, D), np.float32)
    for r in range(N_CORES):
        shard = results[r]["out_shard"]          # [NCHUNK, P, D]
        for c in range(NCHUNK):
            t0 = c * CHUNK + r * P
            out[t0:t0 + P, :] = shard[c]
    return out.reshape(B, L, D)


def kernel(x, W1, b1, W2, b2, Wg, bg, k):
    from concourse.bass_utils import run_bass_kernel_spmd

    assert int(k) == 2
    if "nc" not in _cache:
        _cache["nc"] = _build()
    nc = _cache["nc"]
    in_maps = _host_inputs(np.asarray(x), np.asarray(W1), np.asarray(b1),
                           np.asarray(W2), np.asarray(b2), np.asarray(Wg),
                           np.asarray(bg))
    res = run_bass_kernel_spmd(nc, in_maps, core_ids=list(range(N_CORES)),
                               **_cache.get("run_kwargs", {}))
    _cache["last_result"] = res
    return _assemble(res.results)
